# revision 1
# baseline (speedup 1.0000x reference)
"""Trainium2 Bass kernel for HGNN-MLP (email/url/sender heterograph).

Math (dead-code-eliminated vs the full module: out_url/out_sender are unused):
  out = relu( x_email @ Wer  +  T @ Wcomb[:12] + bias_row )[*, :] @ Wc + bc
where
  Wer      = W_email @ (Wroot_ue + Wroot_se)                       [768,128]
  T[d,0:9]  = sum over ue-edges into d of [x_url[src], 1]           (9 cols)
  T[d,9:11] = sum over se-edges into d of [x_sender[src], 1]        (2 cols)
  Wcomb    = [[W_url;b_url]@Wrel_ue ; [W_sender;b_sender]@Wrel_se]  [11,128]
  bias_row = brel_ue + brel_se + b_email @ (Wroot_ue + Wroot_se)

Distribution: 8-way data-parallel over destination emails (12500/core),
edge lists bucketed by dst partition on host; small weights replicated.
Device does: per-edge gather of 12-float augmented rows (indirect DMA,
128 edges/instruction), scatter-sum via one-hot matmuls accumulated in
PSUM per 128-email stripe, then the big x_email projection fused with the
aggregation term and classifier. No collectives.
"""
import numpy as np
from contextlib import ExitStack

import concourse.bacc as bacc
import concourse.mybir as mybir
from concourse.bass import IndirectOffsetOnAxis
from concourse.bass_utils import run_bass_kernel_spmd

F32 = mybir.dt.float32
I32 = mybir.dt.int32

N_EMAIL, N_URL, N_SENDER = 100000, 400000, 50000
NCORE = 8
EPC = 12500                  # emails per core
NSTR = 98                    # 128-email stripes (12544 >= 12500)
CPS = 25                     # chunks (of 128 edges) per stripe
NCHUNK = NSTR * CPS          # 2450
GRP = 50                     # chunks per pipeline group (2 stripes)
NGRP = NCHUNK // GRP         # 49
SLOTS = 2 * GRP              # ring slots for gather/onehot tiles
NTAB = 450001                # combined table rows (+1 zero row)
ZROW = 450000
EPAD = 12800                 # email cols padded for 25x512 blocks
NBLK, BW = 25, 512

_prog_cache = {}


def _build_program():
    if "nc" in _prog_cache:
        return _prog_cache["nc"]
    nc = bacc.Bacc("TRN2")

    tab = nc.dram_tensor("tab", (NTAB, 12), F32, kind="ExternalInput")
    src = nc.dram_tensor("src", (128, NCHUNK), I32, kind="ExternalInput")
    rel = nc.dram_tensor("rel", (128, NCHUNK), F32, kind="ExternalInput")
    xT = nc.dram_tensor("xT", (768, EPAD), F32, kind="ExternalInput")
    wer = nc.dram_tensor("wer", (768, 128), F32, kind="ExternalInput")
    wcomb = nc.dram_tensor("wcomb", (13, 128), F32, kind="ExternalInput")
    wc = nc.dram_tensor("wc", (128, 2), F32, kind="ExternalInput")
    tabinit = nc.dram_tensor("tabinit", (13, EPAD), F32, kind="ExternalInput")
    out = nc.dram_tensor("out", (2, EPAD), F32, kind="ExternalOutput")

    with ExitStack() as ctx:
        E = ctx.enter_context
        src_sb = E(nc.sbuf_tensor("src_sb", (128, NCHUNK), I32))
        rel_sb = E(nc.sbuf_tensor("rel_sb", (128, NCHUNK), F32))
        iota_sb = E(nc.sbuf_tensor("iota_sb", (128, 128), F32))
        g_sb = E(nc.sbuf_tensor("g_sb", (128, SLOTS * 12), F32))
        oh_sb = E(nc.sbuf_tensor("oh_sb", (128, SLOTS * 128), F32))
        tab_sb = E(nc.sbuf_tensor("tab_sb", (13, EPAD), F32))
        w_sb = E(nc.sbuf_tensor("w_sb", (128, 6 * 128), F32))
        wcomb_sb = E(nc.sbuf_tensor("wcomb_sb", (13, 128), F32))
        wc_sb = E(nc.sbuf_tensor("wc_sb", (128, 2), F32))
        x_sb = E(nc.sbuf_tensor("x_sb", (128, 2 * 6 * BW), F32))
        zr_sb = E(nc.sbuf_tensor("zr_sb", (128, 2 * BW), F32))
        out_sb = E(nc.sbuf_tensor("out_sb", (2, EPAD), F32))

        ps_sc = [E(nc.psum_tensor(f"ps_sc{i}", (12, 128), F32)) for i in range(4)]
        ps_z = [E(nc.psum_tensor(f"ps_z{i}", (128, BW), F32)) for i in range(2)]
        ps_o = [E(nc.psum_tensor(f"ps_o{i}", (2, BW), F32)) for i in range(2)]

        with (
            nc.Block() as block,
            nc.semaphore("ldsem") as ldsem,    # src/rel loads (16 each)
            nc.semaphore("wsem") as wsem,      # weight loads (16 each)
            nc.semaphore("isem") as isem,      # iota + tab memset done
            nc.semaphore("gsemA") as gsemA,    # gather DMAs, even groups
            nc.semaphore("gsemB") as gsemB,    # gather DMAs, odd groups
            nc.semaphore("xsemB") as xsemB,    # x blocks, odd
            nc.semaphore("dvesem") as dvesem,  # onehot groups built
            nc.semaphore("s2sem") as s2sem,    # PE finished stripe accum
            nc.semaphore("addsem") as addsem,  # DVE added stripe to table
            nc.semaphore("xsem") as xsem,      # x blocks, even
            nc.semaphore("zsem") as zsem,      # PE finished z block
            nc.semaphore("rsem") as rsem,      # relu done per block
            nc.semaphore("osem") as osem,      # classifier matmul done
            nc.semaphore("ocop") as ocop,      # out copy done
            nc.semaphore("odma") as odma,      # final store
        ):

            @block.sync
            def _(sy):
                sy.dma_start(out=src_sb[:], in_=src[:]).then_inc(ldsem, 16)
                sy.dma_start(out=rel_sb[:], in_=rel[:]).then_inc(ldsem, 16)
                for k in range(6):
                    sy.dma_start(
                        out=w_sb[:, k * 128:(k + 1) * 128],
                        in_=wer[k * 128:(k + 1) * 128, :],
                    ).then_inc(wsem, 16)
                sy.dma_start(out=wcomb_sb[:], in_=wcomb[:]).then_inc(wsem, 16)
                sy.dma_start(out=wc_sb[:], in_=wc[:]).then_inc(wsem, 16)
                sy.dma_start(out=tab_sb[:], in_=tabinit[:]).then_inc(isem, 16)
                # email feature blocks, ring of 2, runs during scatter phase
                for b in range(NBLK):
                    if b >= 2:
                        sy.wait_ge(zsem, b - 1)
                    for k in range(6):
                        sy.dma_start(
                            out=x_sb[:, (b % 2) * 6 * BW + k * BW:
                                     (b % 2) * 6 * BW + (k + 1) * BW],
                            in_=xT[k * 128:(k + 1) * 128, b * BW:(b + 1) * BW],
                        ).then_inc(xsem if b % 2 == 0 else xsemB, 16)
                sy.wait_ge(ocop, NBLK)
                sy.dma_start(out=out[:], in_=out_sb[:]).then_inc(odma, 16)
                sy.wait_ge(odma, 16)

            @block.gpsimd
            def _(gp):
                gp.iota(
                    iota_sb[:], [[1, 128]], channel_multiplier=0,
                    allow_small_or_imprecise_dtypes=True,
                ).then_inc(isem, 1)
                gp.wait_ge(ldsem, 32)
                for g in range(NGRP):
                    if g >= 2:
                        gp.wait_ge(s2sem, 2 * (g - 1))
                    for k in range(GRP):
                        j = g * GRP + k
                        sl = j % SLOTS
                        gp.indirect_dma_start(
                            out=g_sb[:, sl * 12:(sl + 1) * 12],
                            out_offset=None,
                            in_=tab[:],
                            in_offset=IndirectOffsetOnAxis(
                                ap=src_sb[:, j:j + 1], axis=0
                            ),
                        ).then_inc(gsemA if g % 2 == 0 else gsemB, 16)


            def _ve_email_block(ve, b):
                ve.wait_ge(zsem, b + 1)
                if b >= 2:
                    ve.wait_ge(osem, b - 1)
                ve.tensor_scalar_max(
                    zr_sb[:, (b % 2) * BW:(b % 2 + 1) * BW],
                    ps_z[b % 2][:],
                    0.0,
                ).then_inc(rsem, 1)
                ve.wait_ge(osem, b + 1)
                ve.tensor_copy(
                    out=out_sb[:, b * BW:(b + 1) * BW],
                    in_=ps_o[b % 2][:],
                ).then_inc(ocop, 1)

            @block.vector
            def _(ve):
                ve.wait_ge(ldsem, 32)
                ve.wait_ge(isem, 17)
                for g in range(NGRP):
                    if g >= 2:
                        ve.wait_ge(s2sem, 2 * (g - 1))
                    for k in range(GRP):
                        j = g * GRP + k
                        sl = j % SLOTS
                        inst = ve.tensor_tensor(
                            out=oh_sb[:, sl * 128:(sl + 1) * 128],
                            in0=rel_sb[:, j:j + 1].to_broadcast([128, 128]),
                            in1=iota_sb[:],
                            op=mybir.AluOpType.is_equal,
                        )
                        if k == GRP - 1:
                            inst.then_inc(dvesem, 1)
                    # adds for the two stripes of the previous group
                    if g >= 1:
                        for t in range(2):
                            s = 2 * (g - 1) + t
                            ve.wait_ge(s2sem, s + 1)
                            ve.tensor_add(
                                tab_sb[0:12, s * 128:(s + 1) * 128],
                                tab_sb[0:12, s * 128:(s + 1) * 128],
                                ps_sc[s % 4][:],
                            ).then_inc(addsem, 1)
                    if g >= 3 and (g - 3) % 2 == 0:
                        _ve_email_block(ve, (g - 3) // 2)
                for t in range(2):
                    s = 2 * (NGRP - 1) + t
                    ve.wait_ge(s2sem, s + 1)
                    ve.tensor_add(
                        tab_sb[0:12, s * 128:(s + 1) * 128],
                        tab_sb[0:12, s * 128:(s + 1) * 128],
                        ps_sc[s % 4][:],
                    ).then_inc(addsem, 1)
                # remaining email blocks
                for b in range(23, NBLK):
                    _ve_email_block(ve, b)


            def _pe_email_block(te, b):
                te.wait_ge(addsem, min(4 * (b + 1), NSTR))
                te.wait_ge(xsem if b % 2 == 0 else xsemB,
                           16 * 6 * (b // 2 + 1))
                if b >= 2:
                    te.wait_ge(rsem, b - 1)
                for k in range(6):
                    te.matmul(
                        ps_z[b % 2][:],
                        w_sb[:, k * 128:(k + 1) * 128],
                        x_sb[:, (b % 2) * 6 * BW + k * BW:
                             (b % 2) * 6 * BW + (k + 1) * BW],
                        start=(k == 0),
                        stop=False,
                    )
                te.matmul(
                    ps_z[b % 2][:],
                    wcomb_sb[:],
                    tab_sb[:, b * BW:(b + 1) * BW],
                    start=False,
                    stop=True,
                ).then_inc(zsem, 1)
                te.wait_ge(rsem, b + 1)
                if b >= 2:
                    te.wait_ge(ocop, b - 1)
                te.matmul(
                    ps_o[b % 2][:],
                    wc_sb[:],
                    zr_sb[:, (b % 2) * BW:(b % 2 + 1) * BW],
                    start=True,
                    stop=True,
                ).then_inc(osem, 1)

            @block.tensor
            def _(te):
                te.wait_ge(wsem, 16 * 8)
                for g in range(NGRP):
                    te.wait_ge(dvesem, g + 1)
                    te.wait_ge(gsemA if g % 2 == 0 else gsemB,
                               16 * GRP * (g // 2 + 1))
                    for t in range(2):
                        s = 2 * g + t
                        if s >= 4:
                            te.wait_ge(addsem, s - 3)
                        for k25 in range(CPS):
                            j = s * CPS + k25
                            sl = j % SLOTS
                            inst = te.matmul(
                                ps_sc[s % 4][:],
                                g_sb[:, sl * 12:(sl + 1) * 12],
                                oh_sb[:, sl * 128:(sl + 1) * 128],
                                start=(k25 == 0),
                                stop=(k25 == CPS - 1),
                            )
                            if k25 == CPS - 1:
                                inst.then_inc(s2sem, 1)
                    if g >= 2 and (g - 2) % 2 == 0:
                        _pe_email_block(te, (g - 2) // 2)
                # remaining email blocks
                for b in range(24, NBLK):
                    _pe_email_block(te, b)

    nc.compile()
    _prog_cache["nc"] = nc
    return nc


def _host_prep(inputs):
    f32 = np.float32
    x_email = np.asarray(inputs["x_email"], f32)
    x_url = np.asarray(inputs["x_url"], f32)
    x_sender = np.asarray(inputs["x_sender"], f32)

    # combined augmented table
    tab = np.zeros((NTAB, 12), f32)
    tab[:N_URL, 0:8] = x_url
    tab[:N_URL, 8] = 1.0
    tab[N_URL:N_URL + N_SENDER, 9] = x_sender[:, 0]
    tab[N_URL:N_URL + N_SENDER, 10] = 1.0

    # folded weights
    wroot = inputs["Wroot_ue"] + inputs["Wroot_se"]
    wer = np.ascontiguousarray((inputs["W_email"] @ wroot).astype(f32))
    wcomb = np.zeros((13, 128), f32)
    wcomb[0:8] = inputs["W_url"] @ inputs["Wrel_ue"]
    wcomb[8] = inputs["b_url"] @ inputs["Wrel_ue"]
    wcomb[9] = inputs["W_sender"][0] @ inputs["Wrel_se"]
    wcomb[10] = inputs["b_sender"] @ inputs["Wrel_se"]
    wcomb[12] = (inputs["brel_ue"] + inputs["brel_se"]
                 + inputs["b_email"] @ wroot)
    wc = np.ascontiguousarray(inputs["Wc"].astype(f32))

    # per-core edge buckets: chunk layout [slot(128 part), chunk]
    src_all = np.concatenate([
        np.asarray(inputs["src_ue"], np.int64),
        np.asarray(inputs["src_se"], np.int64) + N_URL,
    ]).astype(np.int32)
    dst_all = np.concatenate([
        np.asarray(inputs["dst_ue"], np.int32),
        np.asarray(inputs["dst_se"], np.int32),
    ])
    core_of = dst_all // EPC

    in_maps = []
    for c in range(NCORE):
        m = core_of == c
        s = src_all[m]
        d = dst_all[m] - c * EPC
        o = np.argsort(d, kind="stable")
        s, d = s[o], d[o]
        bounds = np.searchsorted(d, np.arange(NSTR + 1) * 128)
        SRC = np.full((NCHUNK, 128), ZROW, np.int32)
        REL = np.full((NCHUNK, 128), -1.0, f32)
        for st in range(NSTR):
            a, b = int(bounds[st]), int(bounds[st + 1])
            n = b - a
            assert n <= CPS * 128, f"stripe overflow core {c} stripe {st}: {n}"
            SRC[st * CPS:(st + 1) * CPS].reshape(-1)[:n] = s[a:b]
            REL[st * CPS:(st + 1) * CPS].reshape(-1)[:n] = (
                d[a:b] - st * 128).astype(f32)
        xTc = np.zeros((768, EPAD), f32)
        xTc[:, :EPC] = x_email[c * EPC:(c + 1) * EPC].T
        tabinit_np = np.zeros((13, EPAD), f32)
        tabinit_np[12] = 1.0
        in_maps.append({
            "tab": tab,
            "tabinit": tabinit_np,
            "src": np.ascontiguousarray(SRC.T),
            "rel": np.ascontiguousarray(REL.T),
            "xT": xTc,
            "wer": wer,
            "wcomb": wcomb,
            "wc": wc,
        })
    return in_maps


def kernel(**inputs):
    nc = _build_program()
    in_maps = _host_prep(inputs)
    res = None
    last_exc = None
    for _attempt in range(3):
        try:
            res = run_bass_kernel_spmd(nc, in_maps, list(range(NCORE)))
            break
        except Exception as e:  # transient device wedge recovers on retry
            last_exc = e
            import time as _time
            _time.sleep(5.0)
    if res is None:
        raise last_exc
    out = np.empty((N_EMAIL, 2), np.float32)
    bc = np.asarray(inputs["bc"], np.float32)
    for c in range(NCORE):
        out[c * EPC:(c + 1) * EPC] = res.results[c]["out"][:, :EPC].T
    return out + bc



# revision 7
# speedup vs baseline: 17.7387x; 17.7387x over previous
"""Trainium2 Bass kernel for HGNN-MLP (email/url/sender heterograph).

Math (dead-code-eliminated: out_url/out_sender unused by the return value):
  out = relu( x_email @ Wer + T @ Wcomb )[:, :] @ Wc + bc
where
  Wer  = W_email @ (Wroot_ue + Wroot_se)                      [768,128]
  T[d] = [sum x_url[src] over ue edges, deg_ue, sum x_sender[src]
          over se edges, deg_se, 1]  (12 cols, 8 replicas)
  Wcomb folds W_url@Wrel_ue, b_url@Wrel_ue, W_sender@Wrel_se,
          b_sender@Wrel_se and the bias row.

Distribution: 8-way data-parallel over destination emails (12500/core),
edge lists bucketed by dst partition on host; small weights replicated.

Device strategy per core: batched indirect-DMA gathers of source rows
(url: 8 bf16, sender: 1 bf16) followed by indirect-DMA scatter-ADD into a
DRAM table T with 8 row-replicas per email.  Edges are grouped host-side
into rounds so every scatter instruction has unique destination rows
(required: the DMA compute-op read-modify-write does not accumulate
duplicate indices within one instruction).  The dense phase streams
x_email.T in bf16, accumulates x@Wer into PSUM (spilled to SBUF so it
overlaps the scatter phase), then adds the T@Wcomb term after T is read
back transposed, applies relu and the tiny classifier.  No collectives.
"""
import numpy as np
from contextlib import ExitStack
import ml_dtypes

import concourse.bacc as bacc
import concourse.mybir as mybir
from concourse.bass import IndirectOffsetOnAxis
from concourse.bass_utils import run_bass_kernel_spmd

F32 = mybir.dt.float32
BF16 = mybir.dt.bfloat16
I32 = mybir.dt.int32
BF = ml_dtypes.bfloat16

N_EMAIL, N_URL, N_SENDER = 100000, 400000, 50000
NCORE = 8
EPC = 12500                  # emails per core
EPAD = 12800                 # padded (25 blocks of 512)
NBLK, BW = 25, 512
R = 8                        # scatter row replicas
CAPS_U = [788, 592, 120, 8, 2, 1]   # ue group col caps (measured max +slack)
CAPS_S = [684, 120, 4, 1]           # se group col caps
GU = sum(CAPS_U)             # 1600
GS = sum(CAPS_S)             # 950
NG_U, NG_S = len(CAPS_U), len(CAPS_S)
NG = NG_U + NG_S
UBUF = CAPS_U[0]             # ring slot width (cols) for ue gather buf
SEBUF = CAPS_S[0]
TROWS = EPAD + 16            # 12816 rows of 96 (= EPAD*R + dump rows of 12)
DUMP = EPAD * R              # scatter dump row index (flat [TROWS*8, 12])
XRING = 4                    # x block ring depth
H1B = 12                     # blocks in T half 1
H1C = H1B * BW               # 6144

_prog_cache = {}


def _build_program():
    if "nc" in _prog_cache:
        return _prog_cache["nc"]
    nc = bacc.Bacc("TRN2")

    xT = nc.dram_tensor("xT", (768, EPAD), BF16, kind="ExternalInput")
    url_tab = nc.dram_tensor("url_tab", (N_URL + 1, 8), BF16, kind="ExternalInput")
    snd_tab = nc.dram_tensor("snd_tab", (N_SENDER + 1, 1), BF16, kind="ExternalInput")
    gidx_u = nc.dram_tensor("gidx_u", (128, GU), I32, kind="ExternalInput")
    sidx_u = nc.dram_tensor("sidx_u", (128, GU), I32, kind="ExternalInput")
    gidx_s = nc.dram_tensor("gidx_s", (128, GS), I32, kind="ExternalInput")
    sidx_s = nc.dram_tensor("sidx_s", (128, GS), I32, kind="ExternalInput")
    T = nc.dram_tensor("T", (TROWS, 96), BF16, kind="ExternalInput")
    wer = nc.dram_tensor("wer", (768, 128), BF16, kind="ExternalInput")
    wcomb = nc.dram_tensor("wcomb", (96, 128), BF16, kind="ExternalInput")
    wc = nc.dram_tensor("wc", (128, 2), BF16, kind="ExternalInput")
    ident = nc.dram_tensor("ident", (128, 128), BF16, kind="ExternalInput")
    out = nc.dram_tensor("out", (128, 8 * NBLK), F32, kind="ExternalOutput")

    with ExitStack() as ctx:
        E = ctx.enter_context
        gu_sb = E(nc.sbuf_tensor("gu_sb", (128, 2 * 8 * UBUF), BF16))
        gs_sb = E(nc.sbuf_tensor("gs_sb", (128, 2 * SEBUF), BF16))
        giu_sb = E(nc.sbuf_tensor("giu_sb", (128, GU), I32))
        siu_sb = E(nc.sbuf_tensor("siu_sb", (128, GU), I32))
        gis_sb = E(nc.sbuf_tensor("gis_sb", (128, GS), I32))
        sis_sb = E(nc.sbuf_tensor("sis_sb", (128, GS), I32))
        x_sb = E(nc.sbuf_tensor("x_sb", (128, XRING * 6 * BW), BF16))
        tab_sb = E(nc.sbuf_tensor("tab_sb", (96, EPAD), BF16))
        zx_sb = E(nc.sbuf_tensor("zx_sb", (128, EPAD), BF16))
        zr_sb = E(nc.sbuf_tensor("zr_sb", (128, 2 * BW), BF16))
        wer_sb = E(nc.sbuf_tensor("wer_sb", (128, 768), BF16))
        wcomb_sb = E(nc.sbuf_tensor("wcomb_sb", (96, 128), BF16))
        wc_sb = E(nc.sbuf_tensor("wc_sb", (128, 2), BF16))
        id_sb = E(nc.sbuf_tensor("id_sb", (128, 128), BF16))
        out_sb = E(nc.sbuf_tensor("out_sb", (128, 8 * NBLK), F32))

        ps_z = [E(nc.psum_tensor(f"ps_z{i}", (128, BW), F32)) for i in range(2)]
        ps_z2 = [E(nc.psum_tensor(f"ps_z2{i}", (128, BW), F32)) for i in range(2)]
        ps_o = [E(nc.psum_tensor(f"ps_o{i}", (128, 8), F32)) for i in range(2)]

        # group metadata shared by host prep and device program
        caps = CAPS_U + CAPS_S
        offs_u = np.concatenate([[0], np.cumsum(CAPS_U)]).tolist()
        offs_s = np.concatenate([[0], np.cumsum(CAPS_S)]).tolist()

        sem_names = ["gusem", "susem", "gssem", "sssem", "wsem", "gsA", "gsB",
                     "ssA", "ssB", "xs0", "xs1", "xs2", "xs3", "th1", "th2",
                     "zsem", "zxsem", "z2sem", "rsemA", "rsemB", "osem",
                     "ocsem", "odsem"]
        sems = {n: E(nc.semaphore(n)) for n in sem_names}
        (gusem, susem, gssem, sssem, wsem, gsA, gsB, ssA, ssB,
         xs0, xs1, xs2, xs3, th1, th2, zsem, zxsem, z2sem, rsemA, rsemB,
         osem, ocsem, odsem) = (sems[n] for n in sem_names)
        xsems = [xs0, xs1, xs2, xs3]
        ssems = [ssA, ssB]

        with nc.Block() as block:

            def x_block_dma(e, b):
                sem = xsems[b % XRING]
                if b >= XRING:
                    e.wait_ge(zsem, b - XRING + 1)
                e.dma_start(
                    out=x_sb[:, (b % XRING) * 6 * BW:
                             (b % XRING + 1) * 6 * BW].rearrange(
                        "p (k w) -> p k w", k=6),
                    in_=xT[:, b * BW:(b + 1) * BW].rearrange(
                        "(k p) w -> p k w", k=6),
                ).then_inc(sem, 16)

            @block.sync
            def _(sy):
                sy.dma_start(
                    out=wer_sb[:].rearrange("p (k h) -> p k h", k=6),
                    in_=wer[:].rearrange("(k p) h -> p k h", k=6),
                ).then_inc(wsem, 16)
                sy.dma_start(out=wcomb_sb[:], in_=wcomb[:]).then_inc(wsem, 16)
                sy.dma_start(out=wc_sb[:], in_=wc[:]).then_inc(wsem, 16)
                sy.dma_start(out=id_sb[:], in_=ident[:]).then_inc(wsem, 16)
                for b in range(0, NBLK, 2):
                    x_block_dma(sy, b)
                sy.wait_ge(ssA, 16 * ((NG + 1) // 2))
                sy.wait_ge(ssB, 16 * (NG // 2))
                with nc.allow_non_contiguous_dma(reason="transposed T load"):
                    sy.dma_start(
                        out=tab_sb[:, :H1C],
                        in_=T[0:H1C, :].rearrange("d c -> c d"),
                    ).then_inc(th1, 16)
                sy.wait_ge(ocsem, NBLK)
                sy.dma_start(out=out[:], in_=out_sb[:]).then_inc(odsem, 16)
                sy.wait_ge(odsem, 16)

            @block.scalar
            def _(act):
                act.dma_start(out=giu_sb[:], in_=gidx_u[:]).then_inc(gusem, 16)
                act.dma_start(out=siu_sb[:], in_=sidx_u[:]).then_inc(susem, 16)
                for b in range(1, NBLK, 2):
                    x_block_dma(act, b)
                    if b == 7:
                        act.dma_start(out=gis_sb[:], in_=gidx_s[:]).then_inc(gssem, 16)
                        act.dma_start(out=sis_sb[:], in_=sidx_s[:]).then_inc(sssem, 16)
                act.wait_ge(ssA, 16 * ((NG + 1) // 2))
                act.wait_ge(ssB, 16 * (NG // 2))
                with nc.allow_non_contiguous_dma(reason="transposed T load"):
                    act.dma_start(
                        out=tab_sb[:, H1C:],
                        in_=T[H1C:EPAD, :].rearrange("d c -> c d"),
                    ).then_inc(th2, 16)
                # relu for even blocks
                for b in range(0, NBLK, 2):
                    act.wait_ge(z2sem, b + 1)
                    if b >= 2:
                        act.wait_ge(osem, b - 1)
                    act.activation(
                        zr_sb[:, (b % 2) * BW:(b % 2 + 1) * BW],
                        ps_z2[b % 2][:],
                        mybir.ActivationFunctionType.Relu,
                    ).then_inc(rsemA, 1)

            def gather(gp, i):
                sem = gsA if i % 2 == 0 else gsB
                if i >= 2:
                    gp.wait_ge(ssems[i % 2], 16 * ((i - 2) // 2 + 1))
                if i < NG_U:
                    g = i
                    if g == 0:
                        gp.wait_ge(gusem, 16)
                    k = CAPS_U[g]
                    sl = (g % 2) * 8 * UBUF
                    gp.indirect_dma_start(
                        out=gu_sb[:, sl:sl + 8 * k],
                        out_offset=None,
                        in_=url_tab[:],
                        in_offset=IndirectOffsetOnAxis(
                            ap=giu_sb[:, offs_u[g]:offs_u[g] + k], axis=0),
                    ).then_inc(sem, 16)
                else:
                    g = i - NG_U
                    if g == 0:
                        gp.wait_ge(gssem, 16)
                    k = CAPS_S[g]
                    sl = (g % 2) * SEBUF
                    gp.indirect_dma_start(
                        out=gs_sb[:, sl:sl + k],
                        out_offset=None,
                        in_=snd_tab[:],
                        in_offset=IndirectOffsetOnAxis(
                            ap=gis_sb[:, offs_s[g]:offs_s[g] + k], axis=0),
                    ).then_inc(sem, 16)

            def scatter(gp, i, t_flat):
                sem = gsA if i % 2 == 0 else gsB
                gp.wait_ge(sem, 16 * (i // 2 + 1))
                if i < NG_U:
                    g = i
                    if g == 0:
                        gp.wait_ge(susem, 16)
                    k = CAPS_U[g]
                    sl = (g % 2) * 8 * UBUF
                    gp.indirect_dma_start(
                        out=t_flat,
                        out_offset=IndirectOffsetOnAxis(
                            ap=siu_sb[:, offs_u[g]:offs_u[g] + k], axis=0),
                        in_=gu_sb[:, sl:sl + 8 * k],
                        in_offset=None,
                        compute_op=mybir.AluOpType.add,
                    ).then_inc(ssems[i % 2], 16)
                else:
                    g = i - NG_U
                    if g == 0:
                        gp.wait_ge(sssem, 16)
                    k = CAPS_S[g]
                    sl = (g % 2) * SEBUF
                    gp.indirect_dma_start(
                        out=t_flat,
                        out_offset=IndirectOffsetOnAxis(
                            ap=sis_sb[:, offs_s[g]:offs_s[g] + k], axis=0),
                        in_=gs_sb[:, sl:sl + k],
                        in_offset=None,
                        compute_op=mybir.AluOpType.add,
                        element_offset=9,
                    ).then_inc(ssems[i % 2], 16)

            @block.gpsimd
            def _(gp):
                t_flat = T[:].rearrange("d (r c) -> (d r) c", r=R, c=12)
                gather(gp, 0)
                gather(gp, 1)
                for i in range(NG):
                    scatter(gp, i, t_flat)
                    if i + 2 < NG:
                        gather(gp, i + 2)
                gp.wait_ge(ssA, 16 * ((NG + 1) // 2))
                gp.wait_ge(ssB, 16 * (NG // 2))

            def classifier(te, b):
                if b % 2 == 0:
                    te.wait_ge(rsemA, b // 2 + 1)
                else:
                    te.wait_ge(rsemB, (b - 1) // 2 + 1)
                if b >= 2:
                    te.wait_ge(ocsem, b - 1)
                for j in range(4):
                    inst = te.matmul(
                        ps_o[b % 2][:, 2 * j:2 * j + 2],
                        zr_sb[:, (b % 2) * BW + j * 128:
                              (b % 2) * BW + (j + 1) * 128],
                        wc_sb[:],
                        start=True,
                        stop=True,
                    )
                    if j == 3:
                        inst.then_inc(osem, 1)

            @block.tensor
            def _(te):
                te.wait_ge(wsem, 64)
                # phase a: x @ Wer accumulated per block, spilled by DVE
                for b in range(NBLK):
                    te.wait_ge(xsems[b % XRING], 16 * (b // XRING + 1))
                    if b >= 2:
                        te.wait_ge(zxsem, b - 1)
                    base = (b % XRING) * 6 * BW
                    for k in range(6):
                        inst = te.matmul(
                            ps_z[b % 2][:],
                            wer_sb[:, k * 128:(k + 1) * 128],
                            x_sb[:, base + k * BW:base + (k + 1) * BW],
                            start=(k == 0),
                            stop=(k == 5),
                        )
                        if k == 5:
                            inst.then_inc(zsem, 1)
                # phase b: + zx (identity) + T @ Wcomb, then relu + classifier
                for b in range(NBLK):
                    te.wait_ge(zxsem, b + 1)
                    te.wait_ge(th1 if b < H1B else th2, 16)
                    if b >= 2:
                        if (b - 2) % 2 == 0:
                            te.wait_ge(rsemA, (b - 2) // 2 + 1)
                        else:
                            te.wait_ge(rsemB, (b - 1) // 2)
                    te.matmul(
                        ps_z2[b % 2][:],
                        id_sb[:],
                        zx_sb[:, b * BW:(b + 1) * BW],
                        start=True,
                        stop=False,
                    )
                    te.matmul(
                        ps_z2[b % 2][:],
                        wcomb_sb[:],
                        tab_sb[:, b * BW:(b + 1) * BW],
                        start=False,
                        stop=True,
                    ).then_inc(z2sem, 1)
                    if b >= 1:
                        classifier(te, b - 1)
                classifier(te, NBLK - 1)

            @block.vector
            def _(ve):
                for b in range(NBLK):
                    ve.wait_ge(zsem, b + 1)
                    ve.tensor_copy(
                        out=zx_sb[:, b * BW:(b + 1) * BW],
                        in_=ps_z[b % 2][:],
                    ).then_inc(zxsem, 1)
                for b in range(NBLK):
                    if b % 2 == 1:
                        ve.wait_ge(z2sem, b + 1)
                        if b >= 2:
                            ve.wait_ge(osem, b - 1)
                        ve.tensor_scalar_max(
                            zr_sb[:, (b % 2) * BW:(b % 2 + 1) * BW],
                            ps_z2[b % 2][:],
                            0.0,
                        ).then_inc(rsemB, 1)
                    if b >= 1:
                        ve.wait_ge(osem, b)
                        ve.tensor_copy(
                            out=out_sb[:, (b - 1) * 8:b * 8],
                            in_=ps_o[(b - 1) % 2][:],
                        ).then_inc(ocsem, 1)
                ve.wait_ge(osem, NBLK)
                ve.tensor_copy(
                    out=out_sb[:, (NBLK - 1) * 8:NBLK * 8],
                    in_=ps_o[(NBLK - 1) % 2][:],
                ).then_inc(ocsem, 1)

    nc.compile()
    _prog_cache["nc"] = nc
    return nc


def _pack_groups(src, dst, caps, gpad, spad):
    """Group edges into rounds of R so each group has unique (dst, rep)."""
    order = np.argsort(dst, kind="stable")
    ds, ss = dst[order], src[order]
    starts = np.searchsorted(ds, np.arange(EPAD + 1))
    ranks = np.arange(len(ds)) - starts[ds]
    rep = ranks % R
    grp = ranks // R
    ncols = sum(caps)
    gidx = np.full((128, ncols), gpad, np.int32)
    sidx = np.full((128, ncols), spad, np.int32)
    if len(ds) and grp.max() >= len(caps):
        raise AssertionError(f"degree overflow: max grp {grp.max()}")
    off = 0
    for g, cap in enumerate(caps):
        m = grp == g
        n = int(m.sum())
        assert n <= 128 * cap, f"group {g} overflow: {n} > {128 * cap}"
        bg = np.full(128 * cap, gpad, np.int32)
        bs = np.full(128 * cap, spad, np.int32)
        bg[:n] = ss[m]
        bs[:n] = ds[m] * R + rep[m]
        gidx[:, off:off + cap] = bg.reshape(128, cap)
        sidx[:, off:off + cap] = bs.reshape(128, cap)
        off += cap
    return gidx, sidx


def _host_prep(inputs):
    f32 = np.float32
    x_email = np.asarray(inputs["x_email"], f32)
    x_url = np.asarray(inputs["x_url"], f32)
    x_sender = np.asarray(inputs["x_sender"], f32)

    url_tab = np.zeros((N_URL + 1, 8), BF)
    url_tab[:N_URL] = x_url.astype(BF)
    snd_tab = np.zeros((N_SENDER + 1, 1), BF)
    snd_tab[:N_SENDER, 0] = x_sender[:, 0].astype(BF)

    wroot = inputs["Wroot_ue"] + inputs["Wroot_se"]
    wer = np.ascontiguousarray((inputs["W_email"] @ wroot)).astype(BF)
    wcomb12 = np.zeros((12, 128), f32)
    wcomb12[0:8] = inputs["W_url"] @ inputs["Wrel_ue"]
    wcomb12[8] = inputs["b_url"] @ inputs["Wrel_ue"]
    wcomb12[9] = inputs["W_sender"][0] @ inputs["Wrel_se"]
    wcomb12[10] = inputs["b_sender"] @ inputs["Wrel_se"]
    wcomb12[11] = (inputs["brel_ue"] + inputs["brel_se"]
                   + inputs["b_email"] @ wroot)
    wcomb96 = np.tile(wcomb12, (R, 1)).astype(BF)
    wc = np.ascontiguousarray(inputs["Wc"]).astype(BF)
    ident = np.eye(128, dtype=BF)

    src_ue = np.asarray(inputs["src_ue"], np.int32)
    dst_ue = np.asarray(inputs["dst_ue"], np.int32)
    src_se = np.asarray(inputs["src_se"], np.int32)
    dst_se = np.asarray(inputs["dst_se"], np.int32)

    in_maps = []
    for c in range(NCORE):
        lo, hi = c * EPC, (c + 1) * EPC
        mu = (dst_ue >= lo) & (dst_ue < hi)
        su, du = src_ue[mu], dst_ue[mu] - lo
        ms = (dst_se >= lo) & (dst_se < hi)
        ss_, ds_ = src_se[ms], dst_se[ms] - lo

        gixu, sixu = _pack_groups(su, du, CAPS_U, N_URL, DUMP)
        gixs, sixs = _pack_groups(ss_, ds_, CAPS_S, N_SENDER, DUMP)

        T0 = np.zeros((TROWS, 96), BF)
        deg_u = np.bincount(du, minlength=EPAD).astype(f32)
        deg_s = np.bincount(ds_, minlength=EPAD).astype(f32)
        T0[:EPAD, 8] = deg_u.astype(BF)
        T0[:EPAD, 10] = deg_s.astype(BF)
        T0[:EPAD, 11] = 1.0

        xTc = np.zeros((768, EPAD), BF)
        xTc[:, :EPC] = x_email[lo:hi].T.astype(BF)

        in_maps.append({
            "xT": xTc,
            "url_tab": url_tab,
            "snd_tab": snd_tab,
            "gidx_u": gixu,
            "sidx_u": sixu,
            "gidx_s": gixs,
            "sidx_s": sixs,
            "T": T0,
            "wer": wer,
            "wcomb": wcomb96,
            "wc": wc,
            "ident": ident,
        })
    return in_maps


def kernel(**inputs):
    nc = _build_program()
    in_maps = _host_prep(inputs)
    res = None
    last_exc = None
    for _attempt in range(3):
        try:
            res = run_bass_kernel_spmd(nc, in_maps, list(range(NCORE)))
            break
        except Exception as e:  # transient device wedge recovers on retry
            last_exc = e
            import time as _time
            _time.sleep(5.0)
    if res is None:
        raise last_exc
    out = np.empty((N_EMAIL, 2), np.float32)
    bc = np.asarray(inputs["bc"], np.float32)
    for c in range(NCORE):
        r = np.asarray(res.results[c]["out"])  # [128, 200]
        full = r.reshape(128, NBLK, 4, 2).transpose(1, 2, 0, 3).reshape(EPAD, 2)
        out[c * EPC:(c + 1) * EPC] = full[:EPC]
    return out + bc
